# revision 16
# baseline (speedup 1.0000x reference)
# MLA (multi-head latent attention) forward on 8 Trainium2 NeuronCores.
#
# Sharding: data-parallel over batch (2) x tensor-parallel over heads (4
# heads/core). Core c handles batch c//4 and heads 4*(c%4)..+4. The small
# latent a-projections are replicated inside each batch group; o_proj is
# computed as per-core partials over the local heads' rows and reduced on
# the host during unsharding.
#
# All tensors are bf16 (weights AND activations; fp32 PSUM accumulation)
# which enables fast-weight-load on the PE, halves DMA traffic, and
# doubles DVE throughput. LayerNorm is restructured so it costs almost
# nothing on-device:
#   * mean-centering is folded into the a-projection weights on the host
#     (wqa @ (I - 11^T/n) makes z mean-centered by construction),
#   * gamma folds into the b-projection weights, beta (zero for this
#     model, but handled) rides the PSUM->SBUF evacuation as an
#     Identity-activation per-partition bias,
#   * the variance is an ACT square pass + ones-column matmul reduction,
#     and 1/sqrt(var+eps) is computed as Exp(-0.5 * Ln(var+eps)) with the
#     Ln on the [1,512] row and the Exp fused into the broadcast-copy
#     that was needed anyway (ACT Rsqrt/Reciprocal are banned in bass and
#     DVE reciprocal on a 1-partition row costs 3.3us).
# The same Ln/Exp trick computes the softmax 1/den. Attention is k-major
# (scores.T = [k_tok, q_tok]) feeding P.T directly into the PV matmul;
# the causal mask is a 0/1 triangle multiply after exp. Each weight panel
# is loaded once and reused for both 512-token chunks back to back, so
# LDWEIGHTS is amortized and HBM weight traffic is read exactly once.
import sys

sys.path.insert(0, "/opt/trn_rl_repo")

import numpy as np

H = 16
DN = 128
DR = 64
DV = 128
QL = 1536
KL = 512
HID = 2048
B = 2
S = 1024
NCORES = 8
TP = 4          # head groups (cores per batch)
HPC = H // TP   # heads per core
EPS = 1e-5
SCALE = 1.0 / float(np.sqrt(DN + DR))

KQ = QL // 128      # 12 q-latent feature tiles
KKV = KL // 128     # 4 kv-latent feature tiles
KX = HID // 128     # 16 x feature tiles
NS = S // 128       # 8 token tiles
MQB = HPC * (DN + DR) // 128   # 6 q_b output tiles (4 nope + 2 rope pairs)
MO = HID // 128     # 16 o_proj output tiles

TRACE = False
_COMPILED = None
HAS_BETA = False   # set by kernel() before _build(); LN betas are zero for
                   # this model, which lets the evacuations be plain copies
                   # (the Identity-with-bias variant costs an extra ACT
                   # table and thrashes the activation-table cache)


def _build():
    import concourse.mybir as mybir
    import concourse.tile as tile
    from concourse import bacc

    F32 = mybir.dt.float32
    F32R = mybir.dt.float32r
    BF16 = mybir.dt.bfloat16
    WDT = BF16
    AF = mybir.ActivationFunctionType

    nc = bacc.Bacc("TRN2", target_bir_lowering=False, debug=False)

    # ---- DRAM tensors (per-core inputs; same shapes on every core) ----
    xT_d = nc.dram_tensor("xT", [KX, 128, S], WDT, kind="ExternalInput")
    wqa_d = nc.dram_tensor("wqa", [KQ, 128, KX, 128], WDT, kind="ExternalInput")
    wkva_d = nc.dram_tensor("wkva", [5, 128, KX, 128], WDT, kind="ExternalInput")
    wqb_d = nc.dram_tensor("wqb", [MQB, 128, KQ, 128], WDT, kind="ExternalInput")
    wkbk_d = nc.dram_tensor("wkbk", [HPC, 128, KKV, 128], WDT, kind="ExternalInput")
    wkbv_d = nc.dram_tensor("wkbv", [128, KKV, HPC * DV], WDT, kind="ExternalInput")
    wo_d = nc.dram_tensor("wo", [MO, 128, HPC, 128], WDT, kind="ExternalInput")
    c128_d = nc.dram_tensor("c128", [128, S], WDT, kind="ExternalInput")
    s128_d = nc.dram_tensor("s128", [128, S], WDT, kind="ExternalInput")
    tri_d = nc.dram_tensor("tri", [128, 128], WDT, kind="ExternalInput")
    ones_d = nc.dram_tensor("ones", [128, 1], WDT, kind="ExternalInput")
    brow_d = nc.dram_tensor("brow", [1, 128], F32R, kind="ExternalInput")
    pswap_d = nc.dram_tensor("pswap", [128, 128], WDT, kind="ExternalInput")
    pdup_d = nc.dram_tensor("pdup", [64, 128], WDT, kind="ExternalInput")
    pdupsw_d = nc.dram_tensor("pdupsw", [64, 128], WDT, kind="ExternalInput")
    cq_d = nc.dram_tensor("cq", [128, MQB], F32, kind="ExternalInput")
    ckv_d = nc.dram_tensor("ckv", [128, HPC], F32, kind="ExternalInput")
    bvc_d = nc.dram_tensor("bvc", [128, HPC], F32, kind="ExternalInput")
    o_d = nc.dram_tensor("o_part", [HID, S], WDT, kind="ExternalOutput")

    CH = (slice(0, 512), slice(512, 1024))  # 512-wide token chunks

    with tile.TileContext(nc) as tc:
        with (
            tc.tile_pool(name="const", bufs=1) as constp,
            tc.tile_pool(name="xt", bufs=1) as xtp,
            tc.tile_pool(name="z", bufs=1) as zp,
            tc.tile_pool(name="wpan", bufs=3) as wp,
            tc.tile_pool(name="wres", bufs=1) as wrp,
            tc.tile_pool(name="sq", bufs=4) as sqp,
            tc.tile_pool(name="rows", bufs=3) as rowp,
            tc.tile_pool(name="act", bufs=1) as actp,
            tc.tile_pool(name="pt", bufs=3) as ptp,
            tc.tile_pool(name="ot", bufs=3) as otp,
            tc.tile_pool(name="mm", bufs=3, space="PSUM") as mmp,
            tc.tile_pool(name="num", bufs=3, space="PSUM") as nump,
            tc.tile_pool(name="rowacc", bufs=2, space="PSUM") as rap,
        ):
            # ---- constants ----
            tri = constp.tile([128, 128], WDT)
            nc.gpsimd.dma_start(out=tri, in_=tri_d.ap())
            ones = constp.tile([128, 1], WDT)
            nc.gpsimd.dma_start(out=ones, in_=ones_d.ap())
            brow = constp.tile([1, 128], F32R)
            nc.gpsimd.dma_start(out=brow, in_=brow_d.ap())
            pswap = constp.tile([128, 128], WDT)
            nc.gpsimd.dma_start(out=pswap, in_=pswap_d.ap())
            pdup = constp.tile([64, 128], WDT)
            nc.gpsimd.dma_start(out=pdup, in_=pdup_d.ap())
            pdupsw = constp.tile([64, 128], WDT)
            nc.gpsimd.dma_start(out=pdupsw, in_=pdupsw_d.ap())
            cq = constp.tile([128, MQB], F32)
            nc.gpsimd.dma_start(out=cq, in_=cq_d.ap())
            ckv = constp.tile([128, HPC], F32)
            nc.gpsimd.dma_start(out=ckv, in_=ckv_d.ap())
            bvc = constp.tile([128, HPC], F32)
            nc.gpsimd.dma_start(out=bvc, in_=bvc_d.ap())
            c_t = constp.tile([128, S], WDT)
            nc.gpsimd.dma_start(out=c_t, in_=c128_d.ap())
            s_t = constp.tile([128, S], WDT)
            nc.gpsimd.dma_start(out=s_t, in_=s128_d.ap())
            eps_t = constp.tile([1, 1], F32)
            nc.vector.memset(eps_t, EPS)

            # persistent (full-width) attention operands
            knope = [actp.tile([128, S], WDT, tag=f"kn{h}", name=f"kn{h}")
                     for h in range(HPC)]
            vt = [actp.tile([128, HPC * DV], WDT, tag=f"v{st}", name=f"v{st}")
                  for st in range(NS)]
            krope = actp.tile([128, S], WDT, tag="krope")
            qfull = [actp.tile([128, S], WDT, tag=f"qf{m}", name=f"qf{m}")
                     for m in range(MQB)]
            attn = [actp.tile([128, S], WDT, tag=f"at{h}", name=f"at{h}")
                    for h in range(HPC)]
            rbkv = actp.tile([128, S], WDT, tag="rbkv")
            rbq = actp.tile([128, S], WDT, tag="rbq")

            # ---- x tiles (full width, both chunks) ----
            xt = []
            for k in range(KX):
                t = xtp.tile([128, S], WDT, tag=f"xt{k}", name=f"xt{k}")
                eng = nc.scalar if k % 2 == 0 else nc.sync
                eng.dma_start(out=t, in_=xT_d.ap()[k])
                xt.append(t)

            # a-projection: m-outer, k-inner; each weight panel serves both
            # 512-token chunks back to back so LDWEIGHTS is amortized. The
            # per-chunk sum-of-squares rows accumulate in their own psum
            # banks (allocated at first use so the 2-deep pool cycles
            # kv rows -> q rows -> softmax denominators in order).
            def aproj(ms, w_dram, arow, nsq, zs, pfx):
                for m in ms:
                    pan = wp.tile([128, KX, 128], WDT, tag="w",
                                  name=f"p{pfx}{m}")
                    nc.sync.dma_start(out=pan, in_=w_dram.ap()[m])
                    z = zp.tile([128, S], WDT, tag=f"z{pfx}{m}",
                                name=f"z{pfx}{m}")
                    zs.append(z)
                    ps = [mmp.tile([128, 512], F32, tag="mm",
                                   name=f"za{pfx}{m}_{c}") for c in range(2)]
                    for k in range(KX):
                        for c in range(2):
                            nc.tensor.matmul(ps[c], pan[:, k, :],
                                             xt[k][:, CH[c]],
                                             start=(k == 0),
                                             stop=(k == KX - 1))
                    nc.scalar.activation(z[:, CH[0]], ps[0], AF.Copy)
                    nc.vector.tensor_copy(z[:, CH[1]], ps[1])
                    if m < nsq:
                        sq0 = sqp.tile([128, 512], WDT, tag="sq",
                                       name=f"sq{pfx}{m}_0")
                        nc.scalar.activation(sq0, ps[0], AF.Square)
                        sq1 = sqp.tile([128, 512], WDT, tag="sq",
                                       name=f"sq{pfx}{m}_1")
                        nc.vector.tensor_mul(sq1, z[:, CH[1]], z[:, CH[1]])
                        nc.tensor.matmul(arow[0], ones, sq0,
                                         start=(m == 0), stop=(m == nsq - 1),
                                         skip_group_check=True)
                        nc.tensor.matmul(arow[1], ones, sq1,
                                         start=(m == 0), stop=(m == nsq - 1),
                                         skip_group_check=True)

            # 1/sqrt(var+eps) broadcast to [128, S]: Ln on the row, then
            # Exp(-0.5 x) fused into the PSUM->SBUF copy of the broadcast.
            def make_rb(arow, nf, rb, pfx):
                for c in range(2):
                    lnrow = rowp.tile([1, 512], F32R, tag="lnrow",
                                      name=f"ln{pfx}{c}")
                    nc.scalar.activation(lnrow, arow[c], AF.Ln,
                                         bias=eps_t, scale=1.0 / nf)
                    rb_ps = mmp.tile([128, 512], F32, tag="mm",
                                     name=f"rb{pfx}{c}")
                    nc.tensor.matmul(rb_ps, brow, lnrow, start=True, stop=True)
                    nc.scalar.activation(rb[:, CH[c]], rb_ps, AF.Exp,
                                         scale=-0.5)

            zkv = []
            arow_kv = [rap.tile([1, 512], F32, tag="row", name=f"arkv{c}")
                       for c in range(2)]
            aproj(range(5), wkva_d, arow_kv, KKV, zkv, "k")
            zq = []
            # first q m-tile gives the PE a dense stream to hide the kv
            # finalize chain (Ln -> broadcast -> Exp -> DVE muls) behind
            arow_q = [rap.tile([1, 512], F32, tag="row", name=f"arq{c}")
                      for c in range(2)]
            aproj([0], wqa_d, arow_q, KQ, zq, "q")
            make_rb(arow_kv, KL, rbkv, "k")
            for k in range(KKV):
                nc.vector.tensor_mul(zkv[k], zkv[k], rbkv)
            aproj(range(1, KQ), wqa_d, arow_q, KQ, zq, "q")

            # kv_b: k_nope rows for the local heads (beta rides the evac)
            kbpans = []
            for m in range(HPC):
                kbp = wrp.tile([128, KKV, 128], WDT, tag=f"pkb{m}")
                nc.gpsimd.dma_start(out=kbp, in_=wkbk_d.ap()[m])
                kbpans.append(kbp)
            wkbv = wrp.tile([128, KKV, HPC * DV], WDT, tag="wkbv")
            nc.gpsimd.dma_start(out=wkbv, in_=wkbv_d.ap())
            for m in range(HPC):
                ps = [mmp.tile([128, 512], F32, tag="mm",
                               name=f"kb{m}_{c}") for c in range(2)]
                for k in range(KKV):
                    for c in range(2):
                        nc.tensor.matmul(ps[c], kbpans[m][:, k, :],
                                         zkv[k][:, CH[c]],
                                         start=(k == 0), stop=(k == KKV - 1))
                for c in range(2):
                    if HAS_BETA:
                        nc.scalar.activation(knope[m][:, CH[c]], ps[c],
                                             AF.Identity, bias=ckv[:, m:m + 1])
                    else:
                        nc.scalar.activation(knope[m][:, CH[c]], ps[c],
                                             AF.Copy)

            # q latents: normalize while the PE streams kv_b / V matmuls
            make_rb(arow_q, QL, rbq, "q")
            for k in range(KQ):
                nc.vector.tensor_mul(zq[k], zq[k], rbq)

            # V (token-major): lhsT = normalized latent slice, moving = wkbv
            for st in range(NS):
                ps = mmp.tile([128, 512], F32, tag="mm", name=f"v{st}")
                for k in range(KKV):
                    nc.tensor.matmul(ps, zkv[k][:, st * 128:(st + 1) * 128],
                                     wkbv[:, k, :],
                                     start=(k == 0), stop=(k == KKV - 1))
                if st % 2 == 0:
                    nc.scalar.activation(vt[st], ps, AF.Copy)
                else:
                    nc.vector.tensor_copy(vt[st], ps)

            # k_rope: zkv tile 4 holds the raw rope rows (not centered, not
            # normalized); duplicate to both 64-halves and rotate
            for c in range(2):
                d_ps = mmp.tile([128, 512], F32, tag="mm", name=f"kd{c}")
                nc.tensor.matmul(d_ps, pdup, zkv[4][0:64, CH[c]],
                                 start=True, stop=True)
                dsw_ps = mmp.tile([128, 512], F32, tag="mm", name=f"kds{c}")
                nc.tensor.matmul(dsw_ps, pdupsw, zkv[4][0:64, CH[c]],
                                 start=True, stop=True)
                t2 = sqp.tile([128, 512], WDT, tag="sq", name=f"kt2{c}")
                nc.vector.tensor_mul(t2, dsw_ps, s_t[:, CH[c]])
                t3 = sqp.tile([128, 512], WDT, tag="sq", name=f"kt3{c}")
                nc.vector.tensor_mul(t3, d_ps, c_t[:, CH[c]])
                nc.vector.tensor_add(krope[:, CH[c]], t3, t2)

            # q_b: head-sliced b-projection on normalized latents
            for m in range(MQB):
                pan = wp.tile([128, KQ, 128], WDT, tag="w", name=f"pqb{m}")
                nc.sync.dma_start(out=pan, in_=wqb_d.ap()[m])
                ps = [mmp.tile([128, 512], F32, tag="mm",
                               name=f"qb{m}_{c}") for c in range(2)]
                for k in range(KQ):
                    for c in range(2):
                        nc.tensor.matmul(ps[c], pan[:, k, :],
                                         zq[k][:, CH[c]],
                                         start=(k == 0), stop=(k == KQ - 1))
                for c in range(2):
                    if HAS_BETA:
                        nc.scalar.activation(qfull[m][:, CH[c]], ps[c],
                                             AF.Identity, bias=cq[:, m:m + 1])
                    else:
                        nc.scalar.activation(qfull[m][:, CH[c]], ps[c],
                                             AF.Copy)

            # rope on the two q pair tiles (in place)
            for i in range(2):
                src = qfull[HPC + i]
                for c in range(2):
                    sw_ps = mmp.tile([128, 512], F32, tag="mm",
                                     name=f"qsw{i}_{c}")
                    nc.tensor.matmul(sw_ps, pswap, src[:, CH[c]],
                                     start=True, stop=True)
                    t2 = sqp.tile([128, 512], WDT, tag="sq", name=f"qt2{i}{c}")
                    nc.vector.tensor_mul(t2, sw_ps, s_t[:, CH[c]])
                    t3 = sqp.tile([128, 512], WDT, tag="sq", name=f"qt3{i}{c}")
                    nc.vector.tensor_mul(t3, src[:, CH[c]], c_t[:, CH[c]])
                    nc.vector.tensor_add(src[:, CH[c]], t3, t2)

            # o_proj weights: fully resident so the two chunks can be
            # processed in separate passes (hides the last finalize)
            wops = []
            for m in range(MO):
                pan = wrp.tile([128, HPC, 128], WDT, tag=f"po{m}")
                nc.gpsimd.dma_start(out=pan, in_=wo_d.ap()[m])
                wops.append(pan)

            # ---- attention (k-major, causal): q-chunk outer, head inner.
            # The divide/finalize of unit n is deferred until after unit
            # n+1's matmul stream so the PE (in-order) never waits on the
            # ACT Ln/Exp chain at a unit boundary.
            pending = None
            for c in range(2):
                for h in range(HPC):
                    base = 64 * (h % 2)
                    qr = qfull[HPC + h // 2]
                    num = nump.tile([128, 512], F32, tag="num",
                                    name=f"num{h}_{c}")
                    den = rap.tile([1, 512], F32, tag="row",
                                   name=f"den{h}_{c}")
                    last_ki = (c * 512 + 511) // 128

                    # software pipeline: block ki+1's score matmuls are
                    # emitted before block ki's num/den matmuls, so the
                    # (in-order) PE streams scores while the ACT exp of the
                    # previous block is still in flight.
                    def scores(ki):
                        q0 = ki * 128
                        lo, hi = max(q0, c * 512), (c + 1) * 512
                        w = hi - lo
                        ps = mmp.tile([128, 512], F32, tag="mm",
                                      name=f"sc{h}_{ki}_{c}")
                        nc.tensor.matmul(ps[:, 0:w],
                                         knope[h][:, q0:q0 + 128],
                                         qfull[h][:, lo:hi],
                                         start=True, stop=False)
                        nc.tensor.matmul(ps[:, 0:w],
                                         krope[base:base + 64, q0:q0 + 128],
                                         qr[base:base + 64, lo:hi],
                                         start=False, stop=True)
                        p = ptp.tile([128, 512], WDT, tag="p",
                                     name=f"p{h}_{ki}_{c}")
                        nc.scalar.activation(p[:, 0:w], ps[:, 0:w], AF.Exp,
                                             scale=SCALE)
                        if lo == q0:  # diagonal block: causal triangle
                            nc.vector.tensor_mul(p[:, 0:128], p[:, 0:128], tri)
                        return p, lo, w

                    def numden(blk, ki):
                        p, lo, w = blk
                        nc.tensor.matmul(num[:, lo - c * 512:512],
                                         vt[ki][:, h * 128:(h + 1) * 128],
                                         p[:, 0:w],
                                         start=(ki == 0), stop=(ki == last_ki),
                                         skip_group_check=True)
                        nc.tensor.matmul(den[:, lo - c * 512:512],
                                         ones, p[:, 0:w],
                                         start=(ki == 0), stop=(ki == last_ki),
                                         skip_group_check=True)

                    prev = None
                    for ki in range(last_ki + 1):
                        blk = scores(ki)
                        if prev is not None:
                            numden(prev, ki - 1)
                        prev = blk
                    numden(prev, last_ki)

                    def finalize(h=h, c=c, num=num, den=den):
                        lnden = rowp.tile([1, 512], F32R, tag="lnrow",
                                          name=f"lnd{h}_{c}")
                        nc.scalar.activation(lnden, den, AF.Ln)
                        rb_ps = mmp.tile([128, 512], F32, tag="mm",
                                         name=f"rb{h}_{c}")
                        nc.tensor.matmul(rb_ps, brow, lnden,
                                         start=True, stop=True)
                        rbs = sqp.tile([128, 512], WDT, tag="sq",
                                       name=f"rbs{h}_{c}")
                        nc.scalar.activation(rbs, rb_ps, AF.Exp, scale=-1.0)
                        nc.vector.tensor_mul(attn[h][:, CH[c]], num, rbs)
                        if HAS_BETA:
                            nc.vector.tensor_scalar_add(attn[h][:, CH[c]],
                                                        attn[h][:, CH[c]],
                                                        bvc[:, h:h + 1])

                    if pending is not None:
                        pending()
                    pending = finalize

            # ---- o_proj partials: all of chunk 0 first (the last attention
            # finalize for chunk 1 hides under its matmul stream), then
            # chunk 1; wo panels are resident so no double DMA.
            for c in range(2):
                for m in range(MO):
                    ps = mmp.tile([128, 512], F32, tag="mm", name=f"op{m}_{c}")
                    for k in range(HPC):
                        nc.tensor.matmul(ps, wops[m][:, k, :],
                                         attn[k][:, CH[c]],
                                         start=(k == 0), stop=(k == HPC - 1))
                    if pending is not None and m == 0 and c == 0:
                        pending()
                        pending = None
                    ot = otp.tile([128, 512], WDT, tag="ot", name=f"o{m}_{c}")
                    if m % 2 == 0:
                        nc.scalar.activation(ot, ps, AF.Copy)
                        nc.sync.dma_start(
                            out=o_d.ap()[m * 128:(m + 1) * 128, CH[c]], in_=ot)
                    else:
                        nc.vector.tensor_copy(ot, ps)
                        nc.scalar.dma_start(
                            out=o_d.ap()[m * 128:(m + 1) * 128, CH[c]], in_=ot)

    nc.compile()
    return nc


def _host_prep(x, w_qkv_a, q_ln_g, q_ln_b, w_q_b, w_kv_a, kv_ln_g, kv_ln_b,
               w_kv_b, w_o, freqs_cos, freqs_sin):
    import ml_dtypes
    f32 = np.float32
    bf16 = ml_dtypes.bfloat16
    x = np.asarray(x, f32)
    w_qkv_a = np.asarray(w_qkv_a, f32)
    w_q_b = np.asarray(w_q_b, f32)
    w_kv_a = np.asarray(w_kv_a, f32)
    w_kv_b = np.asarray(w_kv_b, f32)
    w_o = np.asarray(w_o, f32)
    q_ln_g = np.asarray(q_ln_g, f32)
    q_ln_b = np.asarray(q_ln_b, f32)
    kv_ln_g = np.asarray(kv_ln_g, f32)
    kv_ln_b = np.asarray(kv_ln_b, f32)
    cos = np.asarray(freqs_cos, f32)  # [S, 32]
    sin = np.asarray(freqs_sin, f32)

    # interleaved rope dims -> half-split permutation (even dims then odd)
    rp = np.concatenate([np.arange(0, DR, 2), np.arange(1, DR, 2)])

    # mean-centering folded into the a-projection weights: z = x @ wqa_c is
    # mean-centered over its output features by construction
    wqa = w_qkv_a[:, :QL]
    wqa_c = wqa - wqa.mean(axis=1, keepdims=True)
    # kv a-proj augmented: [centered w_kv_a | rope perm | zero pad]
    wkva = np.zeros((HID, 5 * 128), f32)
    wkva[:, :KL] = w_kv_a[:, :KL] - w_kv_a[:, :KL].mean(axis=1, keepdims=True)
    wkva[:, KL:KL + DR] = w_kv_a[:, KL:][:, rp]

    def panels(w, kt, mt):
        # [K, M] -> [mt, 128, kt, 128]: partition-major so DMA rows are
        # contiguous runs
        return np.ascontiguousarray(
            w.reshape(kt, 128, mt, 128).transpose(2, 1, 0, 3))

    # q_b weights: gamma-folded, per-core head slice, col order:
    # [h0n h1n h2n h3n | h0r h1r | h2r h3r], rope dims half-split
    wqb_g = (w_q_b * q_ln_g[:, None]).reshape(QL, H, DN + DR)
    cq_full = (q_ln_b @ w_q_b).reshape(H, DN + DR)
    wkb_g = (w_kv_b * kv_ln_g[:, None]).reshape(KL, H, DN + DV)
    ckv_full = (kv_ln_b @ w_kv_b).reshape(H, DN + DV)

    c128 = np.tile(cos.T, (4, 1)).astype(f32)                    # [128, S]
    s128 = np.tile(np.vstack([-sin.T, sin.T]), (2, 1)).astype(f32)
    tri = np.triu(np.ones((128, 128), f32))                      # keep q>=k
    ones_col = np.ones((128, 1), f32)
    brow = np.ones((1, 128), f32)
    pswap = np.zeros((128, 128), f32)
    for m in range(128):
        pswap[m ^ 32, m] = 1.0
    pdup = np.zeros((64, 128), f32)
    pdupsw = np.zeros((64, 128), f32)
    for m in range(128):
        pdup[m % 64, m] = 1.0
        pdupsw[(m % 64) ^ 32, m] = 1.0

    in_maps = []
    for core in range(NCORES):
        b = core // TP
        h0 = (core % TP) * HPC
        heads = list(range(h0, h0 + HPC))

        wqb_c = np.zeros((QL, MQB * 128), f32)
        cq_c = np.zeros(MQB * 128, f32)
        for i, h in enumerate(heads):
            wqb_c[:, i * 128:(i + 1) * 128] = wqb_g[:, h, :DN]
            cq_c[i * 128:(i + 1) * 128] = cq_full[h, :DN]
            off = HPC * 128 + i * 64
            wqb_c[:, off:off + 64] = wqb_g[:, h, DN:][:, rp]
            cq_c[off:off + 64] = cq_full[h, DN:][rp]

        wkbk_c = np.zeros((KL, HPC * 128), f32)
        ckv_c = np.zeros(HPC * 128, f32)
        wkbv_c = np.zeros((KL, HPC * 128), f32)
        bv_c = np.zeros(HPC * 128, f32)
        for i, h in enumerate(heads):
            wkbk_c[:, i * 128:(i + 1) * 128] = wkb_g[:, h, :DN]
            ckv_c[i * 128:(i + 1) * 128] = ckv_full[h, :DN]
            wkbv_c[:, i * 128:(i + 1) * 128] = wkb_g[:, h, DN:]
            bv_c[i * 128:(i + 1) * 128] = ckv_full[h, DN:]

        wo_c = w_o.reshape(H, DV, HID)[heads].reshape(HPC * DV, HID)

        wt = bf16
        in_maps.append({
            "xT": np.ascontiguousarray(x[b].T).reshape(KX, 128, S).astype(wt),
            "wqa": panels(wqa_c, KX, KQ).astype(wt),
            "wkva": panels(wkva, KX, 5).astype(wt),
            "wqb": panels(wqb_c, KQ, MQB).astype(wt),
            "wkbk": panels(wkbk_c, KKV, HPC).astype(wt),
            "wkbv": np.ascontiguousarray(
                wkbv_c.reshape(KKV, 128, HPC * 128).transpose(1, 0, 2)
            ).astype(wt),
            "wo": panels(wo_c, HPC, MO).astype(wt),
            "c128": c128.astype(wt), "s128": s128.astype(wt),
            "tri": tri.astype(wt),
            "ones": ones_col.astype(wt), "brow": brow,
            "pswap": pswap.astype(wt), "pdup": pdup.astype(wt),
            "pdupsw": pdupsw.astype(wt),
            "cq": np.ascontiguousarray(cq_c.reshape(MQB, 128).T),
            "ckv": np.ascontiguousarray(ckv_c.reshape(HPC, 128).T),
            "bvc": np.ascontiguousarray(bv_c.reshape(HPC, 128).T),
        })
    return in_maps


def kernel(**inputs):
    global _COMPILED, HAS_BETA
    has_beta = bool(np.any(np.asarray(inputs["q_ln_b"]))
                    or np.any(np.asarray(inputs["kv_ln_b"])))
    if _COMPILED is None or has_beta != HAS_BETA:
        HAS_BETA = has_beta
        _COMPILED = _build()
    nc = _COMPILED
    in_maps = _host_prep(**inputs)
    from concourse.bass_utils import run_bass_kernel_spmd
    res = run_bass_kernel_spmd(nc, in_maps, core_ids=list(range(NCORES)),
                               trace=TRACE)
    kernel.last_results = res
    out = np.empty((B, S, HID), np.float32)
    for b in range(B):
        acc = res.results[b * TP]["o_part"].astype(np.float32)
        for t in range(1, TP):
            acc += res.results[b * TP + t]["o_part"].astype(np.float32)
        out[b] = acc.T
    return out


# revision 17
# speedup vs baseline: 1.1799x; 1.1799x over previous
# MLA (multi-head latent attention) forward on 8 Trainium2 NeuronCores.
#
# Sharding: data-parallel over batch (2) x tensor-parallel over heads (4
# heads/core). Core c handles batch c//4 and heads 4*(c%4)..+4. The small
# latent a-projections are replicated inside each batch group; o_proj is
# computed as per-core partials over the local heads' rows and reduced on
# the host during unsharding.
#
# All tensors are bf16 (weights AND activations; fp32 PSUM accumulation)
# which enables fast-weight-load on the PE, halves DMA traffic, and
# doubles DVE throughput. LayerNorm is restructured so it costs almost
# nothing on-device:
#   * mean-centering is folded into the a-projection weights on the host
#     (wqa @ (I - 11^T/n) makes z mean-centered by construction),
#   * gamma folds into the b-projection weights, beta (zero for this
#     model, but handled) rides the PSUM->SBUF evacuation as an
#     Identity-activation per-partition bias,
#   * the variance is an ACT square pass + ones-column matmul reduction,
#     and 1/sqrt(var+eps) is computed as Exp(-0.5 * Ln(var+eps)) with the
#     Ln on the [1,512] row and the Exp fused into the broadcast-copy
#     that was needed anyway (ACT Rsqrt/Reciprocal are banned in bass and
#     DVE reciprocal on a 1-partition row costs 3.3us).
# The same Ln/Exp trick computes the softmax 1/den. Attention is k-major
# (scores.T = [k_tok, q_tok]) feeding P.T directly into the PV matmul;
# the causal mask is a 0/1 triangle multiply after exp. Each weight panel
# is loaded once and reused for both 512-token chunks back to back, so
# LDWEIGHTS is amortized and HBM weight traffic is read exactly once.
import sys

sys.path.insert(0, "/opt/trn_rl_repo")

import numpy as np

H = 16
DN = 128
DR = 64
DV = 128
QL = 1536
KL = 512
HID = 2048
B = 2
S = 1024
NCORES = 8
TP = 4          # head groups (cores per batch)
HPC = H // TP   # heads per core
EPS = 1e-5
SCALE = 1.0 / float(np.sqrt(DN + DR))

KQ = QL // 128      # 12 q-latent feature tiles
KKV = KL // 128     # 4 kv-latent feature tiles
KX = HID // 128     # 16 x feature tiles
NS = S // 128       # 8 token tiles
MQB = HPC * (DN + DR) // 128   # 6 q_b output tiles (4 nope + 2 rope pairs)
MO = HID // 128     # 16 o_proj output tiles

TRACE = False
_COMPILED = None
HAS_BETA = False   # set by kernel() before _build(); LN betas are zero for
                   # this model, which lets the evacuations be plain copies
                   # (the Identity-with-bias variant costs an extra ACT
                   # table and thrashes the activation-table cache)


def _build():
    import concourse.mybir as mybir
    import concourse.tile as tile
    from concourse import bacc

    F32 = mybir.dt.float32
    F32R = mybir.dt.float32r
    BF16 = mybir.dt.bfloat16
    WDT = BF16
    AF = mybir.ActivationFunctionType

    nc = bacc.Bacc("TRN2", target_bir_lowering=False, debug=False)

    # ---- DRAM tensors (per-core inputs; same shapes on every core) ----
    xT_d = nc.dram_tensor("xT", [KX, 128, S], WDT, kind="ExternalInput")
    wqa_d = nc.dram_tensor("wqa", [KQ, 128, KX, 128], WDT, kind="ExternalInput")
    wkva_d = nc.dram_tensor("wkva", [5, 128, KX, 128], WDT, kind="ExternalInput")
    wqb_d = nc.dram_tensor("wqb", [MQB, 128, KQ, 128], WDT, kind="ExternalInput")
    wkbk_d = nc.dram_tensor("wkbk", [HPC, 128, KKV, 128], WDT, kind="ExternalInput")
    wkbv_d = nc.dram_tensor("wkbv", [128, KKV, HPC * DV], WDT, kind="ExternalInput")
    wo_d = nc.dram_tensor("wo", [MO, 128, HPC, 128], WDT, kind="ExternalInput")
    c128_d = nc.dram_tensor("c128", [128, S], WDT, kind="ExternalInput")
    s128_d = nc.dram_tensor("s128", [128, S], WDT, kind="ExternalInput")
    tri_d = nc.dram_tensor("tri", [128, 128], WDT, kind="ExternalInput")
    ones_d = nc.dram_tensor("ones", [128, 1], WDT, kind="ExternalInput")
    brow_d = nc.dram_tensor("brow", [1, 128], F32R, kind="ExternalInput")
    pswap_d = nc.dram_tensor("pswap", [128, 128], WDT, kind="ExternalInput")
    pdup_d = nc.dram_tensor("pdup", [64, 128], WDT, kind="ExternalInput")
    pdupsw_d = nc.dram_tensor("pdupsw", [64, 128], WDT, kind="ExternalInput")
    cq_d = nc.dram_tensor("cq", [128, MQB], F32, kind="ExternalInput")
    ckv_d = nc.dram_tensor("ckv", [128, HPC], F32, kind="ExternalInput")
    bvc_d = nc.dram_tensor("bvc", [128, HPC], F32, kind="ExternalInput")
    o_d = nc.dram_tensor("o_part", [HID, S], WDT, kind="ExternalOutput")

    CH = (slice(0, 512), slice(512, 1024))  # 512-wide token chunks

    with tile.TileContext(nc) as tc:
        with (
            tc.tile_pool(name="const", bufs=1) as constp,
            tc.tile_pool(name="xt", bufs=1) as xtp,
            tc.tile_pool(name="z", bufs=1) as zp,
            tc.tile_pool(name="wpan", bufs=3) as wp,
            tc.tile_pool(name="wres", bufs=1) as wrp,
            tc.tile_pool(name="sq", bufs=4) as sqp,
            tc.tile_pool(name="rows", bufs=3) as rowp,
            tc.tile_pool(name="act", bufs=1) as actp,
            tc.tile_pool(name="pt", bufs=3) as ptp,
            tc.tile_pool(name="ot", bufs=3) as otp,
            tc.tile_pool(name="mm", bufs=3, space="PSUM") as mmp,
            tc.tile_pool(name="num", bufs=3, space="PSUM") as nump,
            tc.tile_pool(name="rowacc", bufs=2, space="PSUM") as rap,
        ):
            # ---- constants ----
            tri = constp.tile([128, 128], WDT)
            nc.gpsimd.dma_start(out=tri, in_=tri_d.ap())
            ones = constp.tile([128, 1], WDT)
            nc.gpsimd.dma_start(out=ones, in_=ones_d.ap())
            brow = constp.tile([1, 128], F32R)
            nc.gpsimd.dma_start(out=brow, in_=brow_d.ap())
            pswap = constp.tile([128, 128], WDT)
            nc.gpsimd.dma_start(out=pswap, in_=pswap_d.ap())
            pdup = constp.tile([64, 128], WDT)
            nc.gpsimd.dma_start(out=pdup, in_=pdup_d.ap())
            pdupsw = constp.tile([64, 128], WDT)
            nc.gpsimd.dma_start(out=pdupsw, in_=pdupsw_d.ap())
            cq = constp.tile([128, MQB], F32)
            nc.gpsimd.dma_start(out=cq, in_=cq_d.ap())
            ckv = constp.tile([128, HPC], F32)
            nc.gpsimd.dma_start(out=ckv, in_=ckv_d.ap())
            bvc = constp.tile([128, HPC], F32)
            nc.gpsimd.dma_start(out=bvc, in_=bvc_d.ap())
            c_t = constp.tile([128, S], WDT)
            nc.gpsimd.dma_start(out=c_t, in_=c128_d.ap())
            s_t = constp.tile([128, S], WDT)
            nc.gpsimd.dma_start(out=s_t, in_=s128_d.ap())
            eps_t = constp.tile([1, 1], F32)
            nc.vector.memset(eps_t, EPS)

            # persistent (full-width) attention operands
            knope = [actp.tile([128, S], WDT, tag=f"kn{h}", name=f"kn{h}")
                     for h in range(HPC)]
            vt = [actp.tile([128, HPC * DV], WDT, tag=f"v{st}", name=f"v{st}")
                  for st in range(NS)]
            krope = actp.tile([128, S], WDT, tag="krope")
            qfull = [actp.tile([128, S], WDT, tag=f"qf{m}", name=f"qf{m}")
                     for m in range(MQB)]
            attn = [actp.tile([128, S], WDT, tag=f"at{h}", name=f"at{h}")
                    for h in range(HPC)]
            rbkv = actp.tile([128, S], WDT, tag="rbkv")
            rbq = actp.tile([128, S], WDT, tag="rbq")

            # ---- x tiles (full width, both chunks) ----
            xt = []
            for k in range(KX):
                t = xtp.tile([128, S], WDT, tag=f"xt{k}", name=f"xt{k}")
                eng = nc.scalar if k % 2 == 0 else nc.sync
                eng.dma_start(out=t, in_=xT_d.ap()[k])
                xt.append(t)

            # a-projection: m-outer, k-inner; each weight panel serves both
            # 512-token chunks back to back so LDWEIGHTS is amortized. The
            # sum-of-squares matmuls of tile m are deferred until tile m+1's
            # matmul stream has been emitted, so the (in-order) PE never
            # waits on the square ops at a tile boundary. Squares run on the
            # DVE from the evacuated bf16 z so the ACT engine only ever runs
            # Copy/Exp/Ln (a 4th function thrashes the activation-table
            # cache at 1.3us per reload).
            def aproj(w_dram, arow, nmt, nsq, zs, pfx, hooks={}):
                pend = None
                for m in range(nmt):
                    pan = wp.tile([128, KX, 128], WDT, tag="w",
                                  name=f"p{pfx}{m}")
                    nc.sync.dma_start(out=pan, in_=w_dram.ap()[m])
                    z = zp.tile([128, S], WDT, tag=f"z{pfx}{m}",
                                name=f"z{pfx}{m}")
                    zs.append(z)
                    ps = [mmp.tile([128, 512], F32, tag="mm",
                                   name=f"za{pfx}{m}_{c}") for c in range(2)]
                    for k in range(KX):
                        for c in range(2):
                            nc.tensor.matmul(ps[c], pan[:, k, :],
                                             xt[k][:, CH[c]],
                                             start=(k == 0),
                                             stop=(k == KX - 1))
                    nc.scalar.activation(z[:, CH[0]], ps[0], AF.Copy)
                    nc.vector.tensor_copy(z[:, CH[1]], ps[1])
                    if pend is not None:
                        pend()
                        pend = None

                    if m < nsq:
                        def mk_sq(m=m, z=z):
                            for c in range(2):
                                sq = sqp.tile([128, 512], WDT, tag="sq",
                                              name=f"sq{pfx}{m}_{c}")
                                nc.vector.tensor_mul(sq, z[:, CH[c]],
                                                     z[:, CH[c]])
                                nc.tensor.matmul(arow[c], ones, sq,
                                                 start=(m == 0),
                                                 stop=(m == nsq - 1),
                                                 skip_group_check=True)
                        pend = mk_sq
                    if m in hooks:
                        hooks[m]()
                if pend is not None:
                    pend()

            # 1/sqrt(var+eps) broadcast to [128, S]: Ln on the row, then
            # Exp(-0.5 x) fused into the PSUM->SBUF copy of the broadcast.
            def make_rb(arow, nf, rb, pfx):
                for c in range(2):
                    lnrow = rowp.tile([1, 512], F32R, tag="lnrow",
                                      name=f"ln{pfx}{c}")
                    nc.scalar.activation(lnrow, arow[c], AF.Ln,
                                         bias=eps_t, scale=1.0 / nf)
                    rb_ps = mmp.tile([128, 512], F32, tag="mm",
                                     name=f"rb{pfx}{c}")
                    nc.tensor.matmul(rb_ps, brow, lnrow, start=True, stop=True)
                    nc.scalar.activation(rb[:, CH[c]], rb_ps, AF.Exp,
                                         scale=-0.5)

            zkv = []
            arow_kv = [rap.tile([1, 512], F32, tag="row", name=f"arkv{c}")
                       for c in range(2)]
            aproj(wkva_d, arow_kv, 5, KKV, zkv, "k")
            zq = []
            arow_q = [rap.tile([1, 512], F32, tag="row", name=f"arq{c}")
                      for c in range(2)]

            def kv_finalize():
                # runs in the shadow of the first q a-proj tile's stream
                make_rb(arow_kv, KL, rbkv, "k")
                for k in range(KKV):
                    nc.vector.tensor_mul(zkv[k], zkv[k], rbkv)

            aproj(wqa_d, arow_q, KQ, KQ, zq, "q", hooks={0: kv_finalize})

            # kv_b: k_nope rows for the local heads
            kbpans = []
            for m in range(HPC):
                kbp = wrp.tile([128, KKV, 128], WDT, tag=f"pkb{m}")
                nc.gpsimd.dma_start(out=kbp, in_=wkbk_d.ap()[m])
                kbpans.append(kbp)
            wkbv = wrp.tile([128, KKV, HPC * DV], WDT, tag="wkbv")
            nc.gpsimd.dma_start(out=wkbv, in_=wkbv_d.ap())
            for m in range(HPC):
                ps = [mmp.tile([128, 512], F32, tag="mm",
                               name=f"kb{m}_{c}") for c in range(2)]
                for k in range(KKV):
                    for c in range(2):
                        nc.tensor.matmul(ps[c], kbpans[m][:, k, :],
                                         zkv[k][:, CH[c]],
                                         start=(k == 0), stop=(k == KKV - 1))
                for c in range(2):
                    if HAS_BETA:
                        nc.scalar.activation(knope[m][:, CH[c]], ps[c],
                                             AF.Identity, bias=ckv[:, m:m + 1])
                    else:
                        nc.scalar.activation(knope[m][:, CH[c]], ps[c],
                                             AF.Copy)

            # q latents: normalize while the PE streams kv_b / V matmuls
            make_rb(arow_q, QL, rbq, "q")
            for k in range(KQ):
                nc.vector.tensor_mul(zq[k], zq[k], rbq)

            # V (token-major): lhsT = normalized latent slice, moving = wkbv
            for st in range(NS):
                ps = mmp.tile([128, 512], F32, tag="mm", name=f"v{st}")
                for k in range(KKV):
                    nc.tensor.matmul(ps, zkv[k][:, st * 128:(st + 1) * 128],
                                     wkbv[:, k, :],
                                     start=(k == 0), stop=(k == KKV - 1))
                if st % 2 == 0:
                    nc.scalar.activation(vt[st], ps, AF.Copy)
                else:
                    nc.vector.tensor_copy(vt[st], ps)

            # k_rope: zkv tile 4 holds the raw rope rows (not centered, not
            # normalized); duplicate to both 64-halves and rotate
            for c in range(2):
                d_ps = mmp.tile([128, 512], F32, tag="mm", name=f"kd{c}")
                nc.tensor.matmul(d_ps, pdup, zkv[4][0:64, CH[c]],
                                 start=True, stop=True)
                dsw_ps = mmp.tile([128, 512], F32, tag="mm", name=f"kds{c}")
                nc.tensor.matmul(dsw_ps, pdupsw, zkv[4][0:64, CH[c]],
                                 start=True, stop=True)
                t2 = sqp.tile([128, 512], WDT, tag="sq", name=f"kt2{c}")
                nc.vector.tensor_mul(t2, dsw_ps, s_t[:, CH[c]])
                t3 = sqp.tile([128, 512], WDT, tag="sq", name=f"kt3{c}")
                nc.vector.tensor_mul(t3, d_ps, c_t[:, CH[c]])
                nc.vector.tensor_add(krope[:, CH[c]], t3, t2)

            # o_proj weights: fully resident (loaded during earlier phases)
            wops = []
            for m in range(MO):
                pan = wrp.tile([128, HPC, 128], WDT, tag=f"po{m}")
                nc.gpsimd.dma_start(out=pan, in_=wo_d.ap()[m])
                wops.append(pan)

            # ---- q_b tile helper (weight panel reused for both chunks) ----
            def qb_tile(m):
                pan = wp.tile([128, KQ, 128], WDT, tag="w", name=f"pqb{m}")
                nc.sync.dma_start(out=pan, in_=wqb_d.ap()[m])
                ps = [mmp.tile([128, 512], F32, tag="mm",
                               name=f"qb{m}_{c}") for c in range(2)]
                for k in range(KQ):
                    for c in range(2):
                        nc.tensor.matmul(ps[c], pan[:, k, :],
                                         zq[k][:, CH[c]],
                                         start=(k == 0), stop=(k == KQ - 1))
                for c in range(2):
                    if HAS_BETA:
                        nc.scalar.activation(qfull[m][:, CH[c]], ps[c],
                                             AF.Identity, bias=cq[:, m:m + 1])
                    else:
                        nc.scalar.activation(qfull[m][:, CH[c]], ps[c],
                                             AF.Copy)

            def qrope(i):
                src = qfull[HPC + i]
                for c in range(2):
                    sw_ps = mmp.tile([128, 512], F32, tag="mm",
                                     name=f"qsw{i}_{c}")
                    nc.tensor.matmul(sw_ps, pswap, src[:, CH[c]],
                                     start=True, stop=True)
                    t2 = sqp.tile([128, 512], WDT, tag="sq", name=f"qt2{i}{c}")
                    nc.vector.tensor_mul(t2, sw_ps, s_t[:, CH[c]])
                    t3 = sqp.tile([128, 512], WDT, tag="sq", name=f"qt3{i}{c}")
                    nc.vector.tensor_mul(t3, src[:, CH[c]], c_t[:, CH[c]])
                    nc.vector.tensor_add(src[:, CH[c]], t3, t2)

            # ---- attention unit (k-major, causal), software-pipelined so
            # the PE streams block ki+1's scores while ACT runs exp(ki).
            # The divide/finalize of a unit is deferred until the next
            # unit's matmul stream. The causal triangle multiply runs on
            # the (otherwise idle) GpSimd engine.
            att = {"pending": None}

            def attention_unit(c, h):
                base = 64 * (h % 2)
                qr = qfull[HPC + h // 2]
                num = nump.tile([128, 512], F32, tag="num",
                                name=f"num{h}_{c}")
                den = rap.tile([1, 512], F32, tag="row", name=f"den{h}_{c}")
                last_ki = (c * 512 + 511) // 128

                def scores(ki):
                    q0 = ki * 128
                    lo, hi = max(q0, c * 512), (c + 1) * 512
                    w = hi - lo
                    ps = mmp.tile([128, 512], F32, tag="mm",
                                  name=f"sc{h}_{ki}_{c}")
                    nc.tensor.matmul(ps[:, 0:w], knope[h][:, q0:q0 + 128],
                                     qfull[h][:, lo:hi],
                                     start=True, stop=False)
                    nc.tensor.matmul(ps[:, 0:w],
                                     krope[base:base + 64, q0:q0 + 128],
                                     qr[base:base + 64, lo:hi],
                                     start=False, stop=True)
                    p = ptp.tile([128, 512], WDT, tag="p",
                                 name=f"p{h}_{ki}_{c}")
                    nc.scalar.activation(p[:, 0:w], ps[:, 0:w], AF.Exp,
                                         scale=SCALE)
                    if lo == q0:  # diagonal block: causal triangle
                        nc.gpsimd.tensor_mul(p[:, 0:128], p[:, 0:128], tri)
                    return p, lo, w

                def numden(blk, ki):
                    p, lo, w = blk
                    nc.tensor.matmul(num[:, lo - c * 512:512],
                                     vt[ki][:, h * 128:(h + 1) * 128],
                                     p[:, 0:w],
                                     start=(ki == 0), stop=(ki == last_ki),
                                     skip_group_check=True)
                    nc.tensor.matmul(den[:, lo - c * 512:512],
                                     ones, p[:, 0:w],
                                     start=(ki == 0), stop=(ki == last_ki),
                                     skip_group_check=True)

                if att["pending"] is not None:
                    att["pending"]()
                    att["pending"] = None
                prev = None
                for ki in range(last_ki + 1):
                    blk = scores(ki)
                    if prev is not None:
                        numden(prev, ki - 1)
                    prev = blk
                numden(prev, last_ki)

                def finalize(h=h, c=c, num=num, den=den):
                    lnden = rowp.tile([1, 512], F32R, tag="lnrow",
                                      name=f"lnd{h}_{c}")
                    nc.scalar.activation(lnden, den, AF.Ln)
                    rb_ps = mmp.tile([128, 512], F32, tag="mm",
                                     name=f"rb{h}_{c}")
                    nc.tensor.matmul(rb_ps, brow, lnden,
                                     start=True, stop=True)
                    rbs = sqp.tile([128, 512], WDT, tag="sq",
                                   name=f"rbs{h}_{c}")
                    nc.scalar.activation(rbs, rb_ps, AF.Exp, scale=-1.0)
                    nc.vector.tensor_mul(attn[h][:, CH[c]], num, rbs)
                    if HAS_BETA:
                        nc.vector.tensor_scalar_add(attn[h][:, CH[c]],
                                                    attn[h][:, CH[c]],
                                                    bvc[:, h:h + 1])
                att["pending"] = finalize

            def o_tile(m, c):
                ps = mmp.tile([128, 512], F32, tag="mm", name=f"op{m}_{c}")
                for k in range(HPC):
                    nc.tensor.matmul(ps, wops[m][:, k, :], attn[k][:, CH[c]],
                                     start=(k == 0), stop=(k == HPC - 1))
                ot = otp.tile([128, 512], WDT, tag="ot", name=f"o{m}_{c}")
                if m % 2 == 0:
                    nc.scalar.activation(ot, ps, AF.Copy)
                    nc.sync.dma_start(
                        out=o_d.ap()[m * 128:(m + 1) * 128, CH[c]], in_=ot)
                else:
                    nc.vector.tensor_copy(ot, ps)
                    nc.scalar.dma_start(
                        out=o_d.ap()[m * 128:(m + 1) * 128, CH[c]], in_=ot)

            # ---- q_b with attention chunk 0 woven in: the rope pair tiles
            # first (so rotations overlap the nope tiles' streams), then
            # each head's nope tile immediately followed by its chunk-0
            # attention unit (exp/finalize hide under the next tile).
            qb_tile(HPC)
            qb_tile(HPC + 1)
            qrope(0)
            qrope(1)
            for h in range(HPC):
                qb_tile(h)
                attention_unit(0, h)

            # ---- attention chunk 1 woven with o_proj chunk 0 ----
            for h in range(HPC):
                attention_unit(1, h)
                for m in range(4 * h, 4 * h + 4):
                    o_tile(m, 0)
            if att["pending"] is not None:
                att["pending"]()
                att["pending"] = None
            for m in range(MO):
                o_tile(m, 1)

    nc.compile()
    return nc


def _host_prep(x, w_qkv_a, q_ln_g, q_ln_b, w_q_b, w_kv_a, kv_ln_g, kv_ln_b,
               w_kv_b, w_o, freqs_cos, freqs_sin):
    import ml_dtypes
    f32 = np.float32
    bf16 = ml_dtypes.bfloat16
    x = np.asarray(x, f32)
    w_qkv_a = np.asarray(w_qkv_a, f32)
    w_q_b = np.asarray(w_q_b, f32)
    w_kv_a = np.asarray(w_kv_a, f32)
    w_kv_b = np.asarray(w_kv_b, f32)
    w_o = np.asarray(w_o, f32)
    q_ln_g = np.asarray(q_ln_g, f32)
    q_ln_b = np.asarray(q_ln_b, f32)
    kv_ln_g = np.asarray(kv_ln_g, f32)
    kv_ln_b = np.asarray(kv_ln_b, f32)
    cos = np.asarray(freqs_cos, f32)  # [S, 32]
    sin = np.asarray(freqs_sin, f32)

    # interleaved rope dims -> half-split permutation (even dims then odd)
    rp = np.concatenate([np.arange(0, DR, 2), np.arange(1, DR, 2)])

    # mean-centering folded into the a-projection weights: z = x @ wqa_c is
    # mean-centered over its output features by construction
    wqa = w_qkv_a[:, :QL]
    wqa_c = wqa - wqa.mean(axis=1, keepdims=True)
    # kv a-proj augmented: [centered w_kv_a | rope perm | zero pad]
    wkva = np.zeros((HID, 5 * 128), f32)
    wkva[:, :KL] = w_kv_a[:, :KL] - w_kv_a[:, :KL].mean(axis=1, keepdims=True)
    wkva[:, KL:KL + DR] = w_kv_a[:, KL:][:, rp]

    def panels(w, kt, mt):
        # [K, M] -> [mt, 128, kt, 128]: partition-major so DMA rows are
        # contiguous runs
        return np.ascontiguousarray(
            w.reshape(kt, 128, mt, 128).transpose(2, 1, 0, 3))

    # q_b weights: gamma-folded, per-core head slice, col order:
    # [h0n h1n h2n h3n | h0r h1r | h2r h3r], rope dims half-split
    wqb_g = (w_q_b * q_ln_g[:, None]).reshape(QL, H, DN + DR)
    cq_full = (q_ln_b @ w_q_b).reshape(H, DN + DR)
    wkb_g = (w_kv_b * kv_ln_g[:, None]).reshape(KL, H, DN + DV)
    ckv_full = (kv_ln_b @ w_kv_b).reshape(H, DN + DV)

    c128 = np.tile(cos.T, (4, 1)).astype(f32)                    # [128, S]
    s128 = np.tile(np.vstack([-sin.T, sin.T]), (2, 1)).astype(f32)
    tri = np.triu(np.ones((128, 128), f32))                      # keep q>=k
    ones_col = np.ones((128, 1), f32)
    brow = np.ones((1, 128), f32)
    pswap = np.zeros((128, 128), f32)
    for m in range(128):
        pswap[m ^ 32, m] = 1.0
    pdup = np.zeros((64, 128), f32)
    pdupsw = np.zeros((64, 128), f32)
    for m in range(128):
        pdup[m % 64, m] = 1.0
        pdupsw[(m % 64) ^ 32, m] = 1.0

    in_maps = []
    for core in range(NCORES):
        b = core // TP
        h0 = (core % TP) * HPC
        heads = list(range(h0, h0 + HPC))

        wqb_c = np.zeros((QL, MQB * 128), f32)
        cq_c = np.zeros(MQB * 128, f32)
        for i, h in enumerate(heads):
            wqb_c[:, i * 128:(i + 1) * 128] = wqb_g[:, h, :DN]
            cq_c[i * 128:(i + 1) * 128] = cq_full[h, :DN]
            off = HPC * 128 + i * 64
            wqb_c[:, off:off + 64] = wqb_g[:, h, DN:][:, rp]
            cq_c[off:off + 64] = cq_full[h, DN:][rp]

        wkbk_c = np.zeros((KL, HPC * 128), f32)
        ckv_c = np.zeros(HPC * 128, f32)
        wkbv_c = np.zeros((KL, HPC * 128), f32)
        bv_c = np.zeros(HPC * 128, f32)
        for i, h in enumerate(heads):
            wkbk_c[:, i * 128:(i + 1) * 128] = wkb_g[:, h, :DN]
            ckv_c[i * 128:(i + 1) * 128] = ckv_full[h, :DN]
            wkbv_c[:, i * 128:(i + 1) * 128] = wkb_g[:, h, DN:]
            bv_c[i * 128:(i + 1) * 128] = ckv_full[h, DN:]

        wo_c = w_o.reshape(H, DV, HID)[heads].reshape(HPC * DV, HID)

        wt = bf16
        in_maps.append({
            "xT": np.ascontiguousarray(x[b].T).reshape(KX, 128, S).astype(wt),
            "wqa": panels(wqa_c, KX, KQ).astype(wt),
            "wkva": panels(wkva, KX, 5).astype(wt),
            "wqb": panels(wqb_c, KQ, MQB).astype(wt),
            "wkbk": panels(wkbk_c, KKV, HPC).astype(wt),
            "wkbv": np.ascontiguousarray(
                wkbv_c.reshape(KKV, 128, HPC * 128).transpose(1, 0, 2)
            ).astype(wt),
            "wo": panels(wo_c, HPC, MO).astype(wt),
            "c128": c128.astype(wt), "s128": s128.astype(wt),
            "tri": tri.astype(wt),
            "ones": ones_col.astype(wt), "brow": brow,
            "pswap": pswap.astype(wt), "pdup": pdup.astype(wt),
            "pdupsw": pdupsw.astype(wt),
            "cq": np.ascontiguousarray(cq_c.reshape(MQB, 128).T),
            "ckv": np.ascontiguousarray(ckv_c.reshape(HPC, 128).T),
            "bvc": np.ascontiguousarray(bv_c.reshape(HPC, 128).T),
        })
    return in_maps


def kernel(**inputs):
    global _COMPILED, HAS_BETA
    has_beta = bool(np.any(np.asarray(inputs["q_ln_b"]))
                    or np.any(np.asarray(inputs["kv_ln_b"])))
    if _COMPILED is None or has_beta != HAS_BETA:
        HAS_BETA = has_beta
        _COMPILED = _build()
    nc = _COMPILED
    in_maps = _host_prep(**inputs)
    from concourse.bass_utils import run_bass_kernel_spmd
    res = run_bass_kernel_spmd(nc, in_maps, core_ids=list(range(NCORES)),
                               trace=TRACE)
    kernel.last_results = res
    out = np.empty((B, S, HID), np.float32)
    for b in range(B):
        acc = res.results[b * TP]["o_part"].astype(np.float32)
        for t in range(1, TP):
            acc += res.results[b * TP + t]["o_part"].astype(np.float32)
        out[b] = acc.T
    return out


# revision 21
# speedup vs baseline: 1.2368x; 1.0482x over previous
# MLA (multi-head latent attention) forward on 8 Trainium2 NeuronCores.
#
# Sharding: data-parallel over batch (2) x tensor-parallel over heads (4
# heads/core). Core c handles batch c//4 and heads 4*(c%4)..+4. The small
# latent a-projections are replicated inside each batch group; o_proj is
# computed as per-core partials over the local heads' rows and reduced on
# the host during unsharding.
#
# All tensors are bf16 (weights AND activations; fp32 PSUM accumulation)
# which enables fast-weight-load on the PE, halves DMA traffic, and
# doubles DVE throughput. LayerNorm is restructured so it costs almost
# nothing on-device:
#   * mean-centering is folded into the a-projection weights on the host
#     (wqa @ (I - 11^T/n) makes z mean-centered by construction),
#   * gamma folds into the b-projection weights, beta (zero for this
#     model, but handled) rides the PSUM->SBUF evacuation as an
#     Identity-activation per-partition bias,
#   * the variance is an ACT square pass + ones-column matmul reduction,
#     and 1/sqrt(var+eps) is computed as Exp(-0.5 * Ln(var+eps)) with the
#     Ln on the [1,512] row and the Exp fused into the broadcast-copy
#     that was needed anyway (ACT Rsqrt/Reciprocal are banned in bass and
#     DVE reciprocal on a 1-partition row costs 3.3us).
# The same Ln/Exp trick computes the softmax 1/den. Attention is k-major
# (scores.T = [k_tok, q_tok]) feeding P.T directly into the PV matmul;
# the causal mask is a 0/1 triangle multiply after exp. Each weight panel
# is loaded once and reused for both 512-token chunks back to back, so
# LDWEIGHTS is amortized and HBM weight traffic is read exactly once.
import sys

sys.path.insert(0, "/opt/trn_rl_repo")

import numpy as np

H = 16
DN = 128
DR = 64
DV = 128
QL = 1536
KL = 512
HID = 2048
B = 2
S = 1024
NCORES = 8
TP = 4          # head groups (cores per batch)
HPC = H // TP   # heads per core
EPS = 1e-5
SCALE = 1.0 / float(np.sqrt(DN + DR))

KQ = QL // 128      # 12 q-latent feature tiles
KKV = KL // 128     # 4 kv-latent feature tiles
KX = HID // 128     # 16 x feature tiles
NS = S // 128       # 8 token tiles
MQB = HPC * (DN + DR) // 128   # 6 q_b output tiles (4 nope + 2 rope pairs)
MO = HID // 128     # 16 o_proj output tiles

TRACE = False
_COMPILED = None
HAS_BETA = False   # set by kernel() before _build(); LN betas are zero for
                   # this model, which lets the evacuations be plain copies
                   # (the Identity-with-bias variant costs an extra ACT
                   # table and thrashes the activation-table cache)


def _build():
    import concourse.mybir as mybir
    import concourse.tile as tile
    from concourse import bacc

    F32 = mybir.dt.float32
    F32R = mybir.dt.float32r
    BF16 = mybir.dt.bfloat16
    WDT = BF16
    AF = mybir.ActivationFunctionType

    nc = bacc.Bacc("TRN2", target_bir_lowering=False, debug=False)

    # ---- DRAM tensors (per-core inputs; same shapes on every core) ----
    xT_d = nc.dram_tensor("xT", [KX, 128, S], WDT, kind="ExternalInput")
    wqa_d = nc.dram_tensor("wqa", [KQ, 128, KX, 128], WDT, kind="ExternalInput")
    wkva_d = nc.dram_tensor("wkva", [5, 128, KX, 128], WDT, kind="ExternalInput")
    wqb_d = nc.dram_tensor("wqb", [MQB, 128, KQ, 128], WDT, kind="ExternalInput")
    wkbk_d = nc.dram_tensor("wkbk", [HPC, 128, KKV, 128], WDT, kind="ExternalInput")
    wkbv_d = nc.dram_tensor("wkbv", [128, KKV, HPC * DV], WDT, kind="ExternalInput")
    wo_d = nc.dram_tensor("wo", [MO, 128, HPC, 128], WDT, kind="ExternalInput")
    c128_d = nc.dram_tensor("c128", [128, S], WDT, kind="ExternalInput")
    s128_d = nc.dram_tensor("s128", [128, S], WDT, kind="ExternalInput")
    tri_d = nc.dram_tensor("tri", [128, 128], WDT, kind="ExternalInput")
    ones_d = nc.dram_tensor("ones", [128, 1], WDT, kind="ExternalInput")
    brow_d = nc.dram_tensor("brow", [1, 128], F32R, kind="ExternalInput")
    pswap_d = nc.dram_tensor("pswap", [128, 128], WDT, kind="ExternalInput")
    pdup_d = nc.dram_tensor("pdup", [64, 128], WDT, kind="ExternalInput")
    pdupsw_d = nc.dram_tensor("pdupsw", [64, 128], WDT, kind="ExternalInput")
    cq_d = nc.dram_tensor("cq", [128, MQB], F32, kind="ExternalInput")
    ckv_d = nc.dram_tensor("ckv", [128, HPC], F32, kind="ExternalInput")
    bvc_d = nc.dram_tensor("bvc", [128, HPC], F32, kind="ExternalInput")
    o_d = nc.dram_tensor("o_part", [HID, S], WDT, kind="ExternalOutput")

    CH = (slice(0, 512), slice(512, 1024))  # 512-wide token chunks

    with tile.TileContext(nc) as tc:
        with (
            tc.tile_pool(name="const", bufs=1) as constp,
            tc.tile_pool(name="xt", bufs=1) as xtp,
            tc.tile_pool(name="z", bufs=1) as zp,
            tc.tile_pool(name="wpan", bufs=3) as wp,
            tc.tile_pool(name="wres", bufs=1) as wrp,
            tc.tile_pool(name="sq", bufs=4) as sqp,
            tc.tile_pool(name="rows", bufs=3) as rowp,
            tc.tile_pool(name="act", bufs=1) as actp,
            tc.tile_pool(name="pt", bufs=3) as ptp,
            tc.tile_pool(name="ot", bufs=3) as otp,
            tc.tile_pool(name="mm", bufs=4, space="PSUM") as mmp,
            tc.tile_pool(name="num", bufs=2, space="PSUM") as nump,
            tc.tile_pool(name="rowacc", bufs=2, space="PSUM") as rap,
        ):
            # ---- constants ----
            tri = constp.tile([128, 128], WDT)
            nc.gpsimd.dma_start(out=tri, in_=tri_d.ap())
            ones = constp.tile([128, 1], WDT)
            nc.gpsimd.dma_start(out=ones, in_=ones_d.ap())
            brow = constp.tile([1, 128], F32R)
            nc.gpsimd.dma_start(out=brow, in_=brow_d.ap())
            pswap = constp.tile([128, 128], WDT)
            nc.gpsimd.dma_start(out=pswap, in_=pswap_d.ap())
            pdup = constp.tile([64, 128], WDT)
            nc.gpsimd.dma_start(out=pdup, in_=pdup_d.ap())
            pdupsw = constp.tile([64, 128], WDT)
            nc.gpsimd.dma_start(out=pdupsw, in_=pdupsw_d.ap())
            cq = constp.tile([128, MQB], F32)
            nc.gpsimd.dma_start(out=cq, in_=cq_d.ap())
            ckv = constp.tile([128, HPC], F32)
            nc.gpsimd.dma_start(out=ckv, in_=ckv_d.ap())
            bvc = constp.tile([128, HPC], F32)
            nc.gpsimd.dma_start(out=bvc, in_=bvc_d.ap())
            c_t = constp.tile([128, S], WDT)
            nc.gpsimd.dma_start(out=c_t, in_=c128_d.ap())
            s_t = constp.tile([128, S], WDT)
            nc.gpsimd.dma_start(out=s_t, in_=s128_d.ap())
            eps_t = constp.tile([1, 1], F32)
            nc.vector.memset(eps_t, EPS)

            # persistent (full-width) attention operands
            knope = [actp.tile([128, S], WDT, tag=f"kn{h}", name=f"kn{h}")
                     for h in range(HPC)]
            vt = [actp.tile([128, HPC * DV], WDT, tag=f"v{st}", name=f"v{st}")
                  for st in range(NS)]
            krope = actp.tile([128, S], WDT, tag="krope")
            qfull = [actp.tile([128, S], WDT, tag=f"qf{m}", name=f"qf{m}")
                     for m in range(MQB)]
            attn = [actp.tile([128, S], WDT, tag=f"at{h}", name=f"at{h}")
                    for h in range(HPC)]
            rbkv = actp.tile([128, S], WDT, tag="rbkv")
            rbq = actp.tile([128, S], WDT, tag="rbq")

            # ---- x tiles (full width, both chunks). The sync queue is
            # reserved for weight panels so the first a-proj panel's DMA is
            # not stuck behind 4MB of x (that cost a 33us preamble).
            xt = []
            xeng = (nc.scalar, nc.gpsimd)
            for k in range(KX):
                t = xtp.tile([128, S], WDT, tag=f"xt{k}", name=f"xt{k}")
                xeng[k % 2].dma_start(out=t, in_=xT_d.ap()[k])
                xt.append(t)

            # a-projection: m-outer, k-inner; each weight panel serves both
            # 512-token chunks back to back so LDWEIGHTS is amortized. The
            # sum-of-squares matmuls of tile m are deferred until tile m+1's
            # matmul stream has been emitted, so the (in-order) PE never
            # waits on the square ops at a tile boundary. Squares run on the
            # DVE from the evacuated bf16 z so the ACT engine only ever runs
            # Copy/Exp/Ln (a 4th function thrashes the activation-table
            # cache at 1.3us per reload).
            def aproj(w_dram, arow, nmt, nsq, zs, pfx, hooks={}):
                pend = None
                for m in range(nmt):
                    pan = wp.tile([128, KX, 128], WDT, tag="w",
                                  name=f"p{pfx}{m}")
                    nc.sync.dma_start(out=pan, in_=w_dram.ap()[m])
                    z = zp.tile([128, S], WDT, tag=f"z{pfx}{m}",
                                name=f"z{pfx}{m}")
                    zs.append(z)
                    ps = [mmp.tile([128, 512], F32, tag="mm",
                                   name=f"za{pfx}{m}_{c}") for c in range(2)]
                    for k in range(KX):
                        for c in range(2):
                            nc.tensor.matmul(ps[c], pan[:, k, :],
                                             xt[k][:, CH[c]],
                                             start=(k == 0),
                                             stop=(k == KX - 1))
                    nc.scalar.activation(z[:, CH[0]], ps[0], AF.Copy)
                    nc.vector.tensor_copy(z[:, CH[1]], ps[1])
                    if pend is not None:
                        pend()
                        pend = None

                    if m < nsq:
                        def mk_sq(m=m, z=z):
                            for c in range(2):
                                sq = sqp.tile([128, 512], WDT, tag="sq",
                                              name=f"sq{pfx}{m}_{c}")
                                nc.vector.tensor_mul(sq, z[:, CH[c]],
                                                     z[:, CH[c]])
                                nc.tensor.matmul(arow[c], ones, sq,
                                                 start=(m == 0),
                                                 stop=(m == nsq - 1),
                                                 skip_group_check=True)
                        pend = mk_sq
                    if m in hooks:
                        hooks[m]()
                if pend is not None:
                    pend()

            # 1/sqrt(var+eps) broadcast to [128, S]: Ln on the row, then
            # Exp(-0.5 x) fused into the PSUM->SBUF copy of the broadcast.
            def make_rb(arow, nf, rb, pfx):
                for c in range(2):
                    lnrow = rowp.tile([1, 512], F32R, tag="lnrow",
                                      name=f"ln{pfx}{c}")
                    nc.scalar.activation(lnrow, arow[c], AF.Ln,
                                         bias=eps_t, scale=1.0 / nf)
                    rb_ps = mmp.tile([128, 512], F32, tag="mm",
                                     name=f"rb{pfx}{c}")
                    nc.tensor.matmul(rb_ps, brow, lnrow, start=True, stop=True)
                    nc.scalar.activation(rb[:, CH[c]], rb_ps, AF.Exp,
                                         scale=-0.5)

            zkv = []
            arow_kv = [rap.tile([1, 512], F32, tag="row", name=f"arkv{c}")
                       for c in range(2)]
            aproj(wkva_d, arow_kv, 5, KKV, zkv, "k")
            zq = []
            arow_q = [rap.tile([1, 512], F32, tag="row", name=f"arq{c}")
                      for c in range(2)]

            def kv_finalize():
                # runs in the shadow of the first q a-proj tile's stream
                make_rb(arow_kv, KL, rbkv, "k")
                for k in range(KKV):
                    nc.vector.tensor_mul(zkv[k], zkv[k], rbkv)

            aproj(wqa_d, arow_q, KQ, KQ, zq, "q", hooks={0: kv_finalize})

            # kv_b: k_nope rows for the local heads
            kbpans = []
            for m in range(HPC):
                kbp = wrp.tile([128, KKV, 128], WDT, tag=f"pkb{m}")
                nc.gpsimd.dma_start(out=kbp, in_=wkbk_d.ap()[m])
                kbpans.append(kbp)
            wkbv = wrp.tile([128, KKV, HPC * DV], WDT, tag="wkbv")
            nc.gpsimd.dma_start(out=wkbv, in_=wkbv_d.ap())
            for m in range(HPC):
                ps = [mmp.tile([128, 512], F32, tag="mm",
                               name=f"kb{m}_{c}") for c in range(2)]
                for k in range(KKV):
                    for c in range(2):
                        nc.tensor.matmul(ps[c], kbpans[m][:, k, :],
                                         zkv[k][:, CH[c]],
                                         start=(k == 0), stop=(k == KKV - 1))
                for c in range(2):
                    if HAS_BETA:
                        nc.scalar.activation(knope[m][:, CH[c]], ps[c],
                                             AF.Identity, bias=ckv[:, m:m + 1])
                    else:
                        nc.scalar.activation(knope[m][:, CH[c]], ps[c],
                                             AF.Copy)

            # q latents: normalize while the PE streams kv_b / V matmuls
            make_rb(arow_q, QL, rbq, "q")
            for k in range(KQ):
                nc.vector.tensor_mul(zq[k], zq[k], rbq)

            # V (token-major): lhsT = normalized latent slice, moving = wkbv
            for st in range(NS):
                ps = mmp.tile([128, 512], F32, tag="mm", name=f"v{st}")
                for k in range(KKV):
                    nc.tensor.matmul(ps, zkv[k][:, st * 128:(st + 1) * 128],
                                     wkbv[:, k, :],
                                     start=(k == 0), stop=(k == KKV - 1))
                if st % 2 == 0:
                    nc.scalar.activation(vt[st], ps, AF.Copy)
                else:
                    nc.vector.tensor_copy(vt[st], ps)

            # k_rope: zkv tile 4 holds the raw rope rows (not centered, not
            # normalized); duplicate to both 64-halves and rotate
            for c in range(2):
                d_ps = mmp.tile([128, 512], F32, tag="mm", name=f"kd{c}")
                nc.tensor.matmul(d_ps, pdup, zkv[4][0:64, CH[c]],
                                 start=True, stop=True)
                dsw_ps = mmp.tile([128, 512], F32, tag="mm", name=f"kds{c}")
                nc.tensor.matmul(dsw_ps, pdupsw, zkv[4][0:64, CH[c]],
                                 start=True, stop=True)
                t2 = sqp.tile([128, 512], WDT, tag="sq", name=f"kt2{c}")
                nc.vector.tensor_mul(t2, dsw_ps, s_t[:, CH[c]])
                t3 = sqp.tile([128, 512], WDT, tag="sq", name=f"kt3{c}")
                nc.vector.tensor_mul(t3, d_ps, c_t[:, CH[c]])
                nc.vector.tensor_add(krope[:, CH[c]], t3, t2)

            # o_proj weights: fully resident (loaded during earlier phases)
            wops = []
            for m in range(MO):
                pan = wrp.tile([128, HPC, 128], WDT, tag=f"po{m}")
                nc.gpsimd.dma_start(out=pan, in_=wo_d.ap()[m])
                wops.append(pan)

            # ---- q_b tile helper (weight panel reused for both chunks) ----
            def qb_tile(m):
                pan = wp.tile([128, KQ, 128], WDT, tag="w", name=f"pqb{m}")
                nc.sync.dma_start(out=pan, in_=wqb_d.ap()[m])
                ps = [mmp.tile([128, 512], F32, tag="mm",
                               name=f"qb{m}_{c}") for c in range(2)]
                for k in range(KQ):
                    for c in range(2):
                        nc.tensor.matmul(ps[c], pan[:, k, :],
                                         zq[k][:, CH[c]],
                                         start=(k == 0), stop=(k == KQ - 1))
                for c in range(2):
                    if HAS_BETA:
                        nc.scalar.activation(qfull[m][:, CH[c]], ps[c],
                                             AF.Identity, bias=cq[:, m:m + 1])
                    else:
                        nc.scalar.activation(qfull[m][:, CH[c]], ps[c],
                                             AF.Copy)

            def qrope(i):
                src = qfull[HPC + i]
                for c in range(2):
                    sw_ps = mmp.tile([128, 512], F32, tag="mm",
                                     name=f"qsw{i}_{c}")
                    nc.tensor.matmul(sw_ps, pswap, src[:, CH[c]],
                                     start=True, stop=True)
                    t2 = sqp.tile([128, 512], WDT, tag="sq", name=f"qt2{i}{c}")
                    nc.vector.tensor_mul(t2, sw_ps, s_t[:, CH[c]])
                    t3 = sqp.tile([128, 512], WDT, tag="sq", name=f"qt3{i}{c}")
                    nc.vector.tensor_mul(t3, src[:, CH[c]], c_t[:, CH[c]])
                    nc.vector.tensor_add(src[:, CH[c]], t3, t2)

            # ---- attention unit (k-major, causal), software-pipelined so
            # the PE streams block ki+1's scores while ACT runs exp(ki).
            # The divide/finalize of a unit is deferred until the next
            # unit's matmul stream. The causal triangle multiply runs on
            # the (otherwise idle) GpSimd engine.
            att = {"pending": None}

            def attention_unit(c, h):
                base = 64 * (h % 2)
                qr = qfull[HPC + h // 2]
                num = nump.tile([128, 512], F32, tag="num",
                                name=f"num{h}_{c}")
                den = rap.tile([1, 512], F32, tag="row", name=f"den{h}_{c}")
                last_ki = (c * 512 + 511) // 128

                def scores(ki):
                    q0 = ki * 128
                    lo, hi = max(q0, c * 512), (c + 1) * 512
                    w = hi - lo
                    ps = mmp.tile([128, 512], F32, tag="mm",
                                  name=f"sc{h}_{ki}_{c}")
                    nc.tensor.matmul(ps[:, 0:w], knope[h][:, q0:q0 + 128],
                                     qfull[h][:, lo:hi],
                                     start=True, stop=False)
                    nc.tensor.matmul(ps[:, 0:w],
                                     krope[base:base + 64, q0:q0 + 128],
                                     qr[base:base + 64, lo:hi],
                                     start=False, stop=True)
                    p = ptp.tile([128, 512], WDT, tag="p",
                                 name=f"p{h}_{ki}_{c}")
                    nc.scalar.activation(p[:, 0:w], ps[:, 0:w], AF.Exp,
                                         scale=SCALE)
                    if lo == q0:  # diagonal block: causal triangle
                        nc.gpsimd.tensor_mul(p[:, 0:128], p[:, 0:128], tri)
                    return p, lo, w

                def numden(blk, ki):
                    p, lo, w = blk
                    nc.tensor.matmul(num[:, lo - c * 512:512],
                                     vt[ki][:, h * 128:(h + 1) * 128],
                                     p[:, 0:w],
                                     start=(ki == 0), stop=(ki == last_ki),
                                     skip_group_check=True)
                    nc.tensor.matmul(den[:, lo - c * 512:512],
                                     ones, p[:, 0:w],
                                     start=(ki == 0), stop=(ki == last_ki),
                                     skip_group_check=True)

                prev = None
                for ki in range(last_ki + 1):
                    blk = scores(ki)
                    if ki == 1 and att["pending"] is not None:
                        # flush the previous unit's divide here: its Ln was
                        # emitted at that unit's end, so by now the row is
                        # ready and the broadcast matmul slots into a busy
                        # PE stream instead of stalling it.
                        att["pending"]()
                        att["pending"] = None
                    if prev is not None:
                        numden(prev, ki - 1)
                    prev = blk
                numden(prev, last_ki)

                # Ln of the softmax denominator runs on ACT right behind
                # this unit's exps (no PE instruction -> no PE stall)
                lnden = rowp.tile([1, 512], F32R, tag="lnrow",
                                  name=f"lnd{h}_{c}")
                nc.scalar.activation(lnden, den, AF.Ln)

                def finalize(h=h, c=c, num=num, lnden=lnden):
                    rb_ps = mmp.tile([128, 512], F32, tag="mm",
                                     name=f"rb{h}_{c}")
                    nc.tensor.matmul(rb_ps, brow, lnden,
                                     start=True, stop=True)
                    rbs = sqp.tile([128, 512], WDT, tag="sq",
                                   name=f"rbs{h}_{c}")
                    nc.scalar.activation(rbs, rb_ps, AF.Exp, scale=-1.0)
                    nc.vector.tensor_mul(attn[h][:, CH[c]], num, rbs)
                    if HAS_BETA:
                        nc.vector.tensor_scalar_add(attn[h][:, CH[c]],
                                                    attn[h][:, CH[c]],
                                                    bvc[:, h:h + 1])
                att["pending"] = finalize

            def o_tile(m, c):
                ps = mmp.tile([128, 512], F32, tag="mm", name=f"op{m}_{c}")
                for k in range(HPC):
                    nc.tensor.matmul(ps, wops[m][:, k, :], attn[k][:, CH[c]],
                                     start=(k == 0), stop=(k == HPC - 1))
                ot = otp.tile([128, 512], WDT, tag="ot", name=f"o{m}_{c}")
                if m % 2 == 0:
                    nc.scalar.activation(ot, ps, AF.Copy)
                    nc.sync.dma_start(
                        out=o_d.ap()[m * 128:(m + 1) * 128, CH[c]], in_=ot)
                else:
                    nc.vector.tensor_copy(ot, ps)
                    nc.scalar.dma_start(
                        out=o_d.ap()[m * 128:(m + 1) * 128, CH[c]], in_=ot)

            # ---- q_b with attention chunk 0 woven in: the rope pair tiles
            # first (so rotations overlap the nope tiles' streams), then
            # each head's nope tile immediately followed by its chunk-0
            # attention unit (exp/finalize hide under the next tile).
            qb_tile(HPC)
            qb_tile(HPC + 1)
            qrope(0)
            qrope(1)
            for h in range(HPC):
                qb_tile(h)
                attention_unit(0, h)

            # ---- attention chunk 1 woven with o_proj chunk 0 ----
            for h in range(HPC):
                attention_unit(1, h)
                for m in range(4 * h, 4 * h + 4):
                    o_tile(m, 0)
            if att["pending"] is not None:
                att["pending"]()
                att["pending"] = None
            for m in range(MO):
                o_tile(m, 1)

    nc.compile()
    return nc


def _host_prep(x, w_qkv_a, q_ln_g, q_ln_b, w_q_b, w_kv_a, kv_ln_g, kv_ln_b,
               w_kv_b, w_o, freqs_cos, freqs_sin):
    import ml_dtypes
    f32 = np.float32
    bf16 = ml_dtypes.bfloat16
    x = np.asarray(x, f32)
    w_qkv_a = np.asarray(w_qkv_a, f32)
    w_q_b = np.asarray(w_q_b, f32)
    w_kv_a = np.asarray(w_kv_a, f32)
    w_kv_b = np.asarray(w_kv_b, f32)
    w_o = np.asarray(w_o, f32)
    q_ln_g = np.asarray(q_ln_g, f32)
    q_ln_b = np.asarray(q_ln_b, f32)
    kv_ln_g = np.asarray(kv_ln_g, f32)
    kv_ln_b = np.asarray(kv_ln_b, f32)
    cos = np.asarray(freqs_cos, f32)  # [S, 32]
    sin = np.asarray(freqs_sin, f32)

    # interleaved rope dims -> half-split permutation (even dims then odd)
    rp = np.concatenate([np.arange(0, DR, 2), np.arange(1, DR, 2)])

    # mean-centering folded into the a-projection weights: z = x @ wqa_c is
    # mean-centered over its output features by construction
    wqa = w_qkv_a[:, :QL]
    wqa_c = wqa - wqa.mean(axis=1, keepdims=True)
    # kv a-proj augmented: [centered w_kv_a | rope perm | zero pad]
    wkva = np.zeros((HID, 5 * 128), f32)
    wkva[:, :KL] = w_kv_a[:, :KL] - w_kv_a[:, :KL].mean(axis=1, keepdims=True)
    wkva[:, KL:KL + DR] = w_kv_a[:, KL:][:, rp]

    def panels(w, kt, mt):
        # [K, M] -> [mt, 128, kt, 128]: partition-major so DMA rows are
        # contiguous runs
        return np.ascontiguousarray(
            w.reshape(kt, 128, mt, 128).transpose(2, 1, 0, 3))

    # q_b weights: gamma-folded, per-core head slice, col order:
    # [h0n h1n h2n h3n | h0r h1r | h2r h3r], rope dims half-split
    wqb_g = (w_q_b * q_ln_g[:, None]).reshape(QL, H, DN + DR)
    cq_full = (q_ln_b @ w_q_b).reshape(H, DN + DR)
    wkb_g = (w_kv_b * kv_ln_g[:, None]).reshape(KL, H, DN + DV)
    ckv_full = (kv_ln_b @ w_kv_b).reshape(H, DN + DV)

    c128 = np.tile(cos.T, (4, 1)).astype(f32)                    # [128, S]
    s128 = np.tile(np.vstack([-sin.T, sin.T]), (2, 1)).astype(f32)
    tri = np.triu(np.ones((128, 128), f32))                      # keep q>=k
    ones_col = np.ones((128, 1), f32)
    brow = np.ones((1, 128), f32)
    pswap = np.zeros((128, 128), f32)
    for m in range(128):
        pswap[m ^ 32, m] = 1.0
    pdup = np.zeros((64, 128), f32)
    pdupsw = np.zeros((64, 128), f32)
    for m in range(128):
        pdup[m % 64, m] = 1.0
        pdupsw[(m % 64) ^ 32, m] = 1.0

    in_maps = []
    for core in range(NCORES):
        b = core // TP
        h0 = (core % TP) * HPC
        heads = list(range(h0, h0 + HPC))

        wqb_c = np.zeros((QL, MQB * 128), f32)
        cq_c = np.zeros(MQB * 128, f32)
        for i, h in enumerate(heads):
            wqb_c[:, i * 128:(i + 1) * 128] = wqb_g[:, h, :DN]
            cq_c[i * 128:(i + 1) * 128] = cq_full[h, :DN]
            off = HPC * 128 + i * 64
            wqb_c[:, off:off + 64] = wqb_g[:, h, DN:][:, rp]
            cq_c[off:off + 64] = cq_full[h, DN:][rp]

        wkbk_c = np.zeros((KL, HPC * 128), f32)
        ckv_c = np.zeros(HPC * 128, f32)
        wkbv_c = np.zeros((KL, HPC * 128), f32)
        bv_c = np.zeros(HPC * 128, f32)
        for i, h in enumerate(heads):
            wkbk_c[:, i * 128:(i + 1) * 128] = wkb_g[:, h, :DN]
            ckv_c[i * 128:(i + 1) * 128] = ckv_full[h, :DN]
            wkbv_c[:, i * 128:(i + 1) * 128] = wkb_g[:, h, DN:]
            bv_c[i * 128:(i + 1) * 128] = ckv_full[h, DN:]

        wo_c = w_o.reshape(H, DV, HID)[heads].reshape(HPC * DV, HID)

        wt = bf16
        in_maps.append({
            "xT": np.ascontiguousarray(x[b].T).reshape(KX, 128, S).astype(wt),
            "wqa": panels(wqa_c, KX, KQ).astype(wt),
            "wkva": panels(wkva, KX, 5).astype(wt),
            "wqb": panels(wqb_c, KQ, MQB).astype(wt),
            "wkbk": panels(wkbk_c, KKV, HPC).astype(wt),
            "wkbv": np.ascontiguousarray(
                wkbv_c.reshape(KKV, 128, HPC * 128).transpose(1, 0, 2)
            ).astype(wt),
            "wo": panels(wo_c, HPC, MO).astype(wt),
            "c128": c128.astype(wt), "s128": s128.astype(wt),
            "tri": tri.astype(wt),
            "ones": ones_col.astype(wt), "brow": brow,
            "pswap": pswap.astype(wt), "pdup": pdup.astype(wt),
            "pdupsw": pdupsw.astype(wt),
            "cq": np.ascontiguousarray(cq_c.reshape(MQB, 128).T),
            "ckv": np.ascontiguousarray(ckv_c.reshape(HPC, 128).T),
            "bvc": np.ascontiguousarray(bv_c.reshape(HPC, 128).T),
        })
    return in_maps


def kernel(**inputs):
    global _COMPILED, HAS_BETA
    has_beta = bool(np.any(np.asarray(inputs["q_ln_b"]))
                    or np.any(np.asarray(inputs["kv_ln_b"])))
    if _COMPILED is None or has_beta != HAS_BETA:
        HAS_BETA = has_beta
        _COMPILED = _build()
    nc = _COMPILED
    in_maps = _host_prep(**inputs)
    from concourse.bass_utils import run_bass_kernel_spmd
    res = run_bass_kernel_spmd(nc, in_maps, core_ids=list(range(NCORES)),
                               trace=TRACE)
    kernel.last_results = res
    out = np.empty((B, S, HID), np.float32)
    for b in range(B):
        acc = res.results[b * TP]["o_part"].astype(np.float32)
        for t in range(1, TP):
            acc += res.results[b * TP + t]["o_part"].astype(np.float32)
        out[b] = acc.T
    return out
